# revision 5
# baseline (speedup 1.0000x reference)
"""Trainium2 Bass kernel for nn_CLUBCategorical (CLUB categorical loss).

Reference computation:
    h      = relu(x @ W1 + b1)              [N, H]
    logits = h @ W2 + b2                    [N, Y]
    logp   = log_softmax(logits, -1)        [N, Y]
    out[i] = logp[i, y_i] - mean_j logp[i, y_j]

Algebraic simplification used here: with c[y] = histogram(y_idx) the
log-softmax normalizer cancels between the positive and negative terms:

    out[i] = L[i, y_i] - (1/N) * (L[i, :] @ c) + (b2[y_i] - (b2 @ c)/N)

where L = relu(x @ W1 + b1) @ W2 (no bias, no softmax). On device this is
two dense matmuls plus a masked column reduction:

    out[i] = sum_y L[i, y] * (onehot(y_i)[y] - c[y]/N) + g[i]

Sharding: data-parallel over N. Each of the 8 cores handles 1024 rows and
gets the full W1/W2 plus the global label histogram (the "all-gather of
column labels" is precomputed on host into c). No collectives needed.

Device layout (per core, everything transposed so the contraction dim is
on SBUF partitions):
    phase 1: hT[j]   [128h, 1024r] = W1[k,jslice].T @ xT[k, rows]  (+b1, relu)
    phase 2: psum_l  [128y,  512r] = W2[j,qslice].T @ hT[j, rows]
             eqc     [128y,  512r] = (ybc == iota_q) - cN_q        (DVE)
             prod    = psum_l * eqc                                (DVE)
             out     += ones.T @ prod  (M=1 matmul, reduce over y) (PE)
Matmuls run in float32r (~TF32 precision, 4x faster than fp32 on PE).
"""

import numpy as np

N, X_DIM, Y_DIM, HIDDEN = 8192, 512, 512, 1024
N_CORES = 8
N_LOC = N // N_CORES          # 1024 rows per core
KX = X_DIM // 128             # 4  k-chunks, phase 1
KH = HIDDEN // 128            # 8  k-chunks, phase 2 / m-chunks, phase 1
QY = Y_DIM // 128             # 4  y-chunks, phase 2
RG = N_LOC // 512             # 2  row groups of 512

_NC_CACHE = {}


def _build(nc_cls, mybir, tile):
    mdt = mybir.dt
    f32 = mdt.float32
    F32R = mdt.float32r
    AF = mybir.ActivationFunctionType
    OP = mybir.AluOpType

    nc = nc_cls("TRN2", target_bir_lowering=False, debug=False,
                num_devices=N_CORES)

    xT = nc.dram_tensor("xT", [X_DIM, N_LOC], f32, kind="ExternalInput")
    W1 = nc.dram_tensor("W1", [X_DIM, HIDDEN], f32, kind="ExternalInput")
    W2 = nc.dram_tensor("W2", [HIDDEN, Y_DIM], f32, kind="ExternalInput")
    # packed constants: [b1c(8) | iot(4) | cNc(4) | ones(1)] = [128, 17]
    cst = nc.dram_tensor("cst", [128, KH + 2 * QY + 1], f32,
                         kind="ExternalInput")
    ybc = nc.dram_tensor("ybc", [128, N_LOC], f32, kind="ExternalInput")
    gv = nc.dram_tensor("gv", [1, N_LOC], f32, kind="ExternalInput")
    out = nc.dram_tensor("out", [1, N_LOC], f32, kind="ExternalOutput")

    with tile.TileContext(nc) as tc:
        with (
            tc.tile_pool(name="wgt", bufs=1) as wgt,
            tc.tile_pool(name="hp", bufs=1) as hp,
            tc.tile_pool(name="eqp", bufs=1) as eqp,
            tc.tile_pool(name="prp", bufs=4) as prp,
            tc.tile_pool(name="osb", bufs=1) as osb,
            tc.tile_pool(name="ph", bufs=3, space="PSUM") as ph,
            tc.tile_pool(name="pl", bufs=3, space="PSUM") as pl,
            tc.tile_pool(name="po", bufs=1, space="PSUM") as po,
        ):
            # DMA issue is serialized per DGE queue. Priority order:
            # phase-1 operands first (xT on sync HWDGE, W1 on scalar
            # HWDGE, interleaved by k so MMs start early); W2 follows W1
            # on the scalar queue (needed only when phase 2 starts);
            # small constants on the gpsimd SWDGE queue.
            cst_sb = wgt.tile([128, KH + 2 * QY + 1], F32R, tag="cst")
            nc.gpsimd.dma_start(cst_sb[:], cst.ap().bitcast(F32R))
            b1_sb = cst_sb[:, 0:KH].bitcast(f32)
            iot_sb = cst_sb[:, KH:KH + QY].bitcast(f32)
            cnc_sb = cst_sb[:, KH + QY:KH + 2 * QY].bitcast(f32)
            ones_sb = cst_sb[:, KH + 2 * QY:KH + 2 * QY + 1]
            g_sb = wgt.tile([1, N_LOC], f32, tag="g")
            nc.gpsimd.dma_start(g_sb[:], gv.ap())
            # Full-K column slices: one descriptor delivers ALL k-chunks
            # for a row-group (xT) or an m-pair (W1), so phase-1 groups
            # unlock after ~1.5MB instead of the full 4MB.
            # xt_sb[n] [128, KX, 512]: xT columns for row-group n, all K.
            xt_sb = []
            for n in range(RG):
                xtt = wgt.tile([128, KX * 512], F32R, tag=f"xt_{n}",
                               name=f"xt_{n}")
                nc.sync.dma_start(
                    xtt[:].rearrange("p (k r) -> p k r", k=KX),
                    xT.ap()[:, n * 512:(n + 1) * 512]
                    .rearrange("(k p) r -> p k r", p=128).bitcast(F32R))
                xt_sb.append(xtt)
            # w1_sb[mp] [128, KX, 256]: W1 columns for m-pair mp, all K.
            w1p_sb = []
            for mp in range(KH // 2):
                w1t = wgt.tile([128, KX * 256], F32R, tag=f"w1_{mp}",
                               name=f"w1_{mp}")
                nc.scalar.dma_start(
                    w1t[:].rearrange("p (k m) -> p k m", k=KX),
                    W1.ap()[:, mp * 256:(mp + 1) * 256]
                    .rearrange("(k p) m -> p k m", p=128).bitcast(F32R))
                w1p_sb.append(w1t)

            def w1_slice(k, m):
                mp, mo = m // 2, m % 2
                return w1p_sb[mp][:, k * 256 + mo * 128:k * 256 + (mo + 1) * 128]

            def xt_slice(k, n):
                return xt_sb[n][:, k * 512:(k + 1) * 512]

            # W2 halves: one per HWDGE queue, right after the phase-1 loads
            w2p_sb = []
            for h in range(2):
                w2p = wgt.tile([128, 4 * Y_DIM], F32R, tag=f"w2p_{h}",
                               name=f"w2p_{h}")
                eng = nc.scalar if h == 0 else nc.sync
                eng.dma_start(
                    w2p[:].rearrange("p (a y) -> p a y", a=4),
                    W2.ap()[h * 512:(h + 1) * 512, :]
                    .rearrange("(a p) y -> p a y", p=128).bitcast(F32R))
                w2p_sb.append(w2p)
            w2_sb = [w2p_sb[j // 4][:, (j % 4) * Y_DIM:(j % 4 + 1) * Y_DIM]
                     for j in range(KH)]
            ybc_sb = wgt.tile([128, N_LOC], f32, tag="ybc")
            nc.gpsimd.dma_start(ybc_sb[:], ybc.ap())

            # --- eqc masks (DVE; no matmul dependency, fills DVE idle time)
            eqc_sb = {}
            for n in range(RG):
                for q in range(QY):
                    e = eqp.tile([128, 512], f32, tag=f"eqc_{n}_{q}")
                    nc.vector.tensor_scalar(
                        e[:], ybc_sb[:, n * 512:(n + 1) * 512],
                        iot_sb[:, q:q + 1], cnc_sb[:, q:q + 1],
                        OP.is_equal, OP.subtract)
                    eqc_sb[(n, q)] = e

            # --- phase 1: hT[j] = relu(W1.T @ xT + b1) ---
            # n-outer so groups complete (and relu) as soon as the m-pair
            # column slice of W1 lands; relus stay spread out on ACT.
            hT = [hp.tile([128, N_LOC], F32R, tag=f"h_{j}", name=f"h_{j}")
                  for j in range(KH)]
            for n in range(RG):
                for m in range(KH):
                    psum = ph.tile([128, 512], f32)
                    for k in range(KX):
                        nc.tensor.matmul(
                            psum[:],
                            w1_slice(k, m),
                            xt_slice(k, n),
                            start=(k == 0), stop=(k == KX - 1))
                    nc.scalar.activation(
                        hT[m][:, n * 512:(n + 1) * 512], psum[:],
                        AF.Relu, bias=b1_sb[:, m:m + 1])

            # --- phase 2 ---
            pout = {n: po.tile([1, 512], f32, tag=f"po_{n}", name=f"po_{n}")
                    for n in range(RG)}
            pending = []  # delay ones-MMs one (n,q) step so PE never waits on DVE

            def flush_one():
                n, q, prod = pending.pop(0)
                nc.tensor.matmul(
                    pout[n][:], ones_sb, prod[:],
                    start=(q == 0), stop=(q == QY - 1))

            for n in range(RG):
                for q in range(QY):
                    psum_l = pl.tile([128, 512], f32)
                    for j in range(KH):
                        nc.tensor.matmul(
                            psum_l[:],
                            w2_sb[j][:, q * 128:(q + 1) * 128],
                            hT[j][:, n * 512:(n + 1) * 512],
                            start=(j == 0), stop=(j == KH - 1))
                    prod = prp.tile([128, 512], F32R)
                    nc.vector.tensor_tensor(
                        prod[:], psum_l[:], eqc_sb[(n, q)][:], OP.mult)
                    pending.append((n, q, prod))
                    if len(pending) >= 2:
                        flush_one()
            while pending:
                flush_one()

            # --- epilogue: add g, store (single output DMA) ---
            o = osb.tile([1, N_LOC], f32, tag="o")
            for n in range(RG):
                nc.vector.tensor_tensor(
                    o[:, n * 512:(n + 1) * 512], pout[n][:],
                    g_sb[:, n * 512:(n + 1) * 512], OP.add)
            nc.sync.dma_start(out.ap(), o[:])

    nc.compile()
    return nc


def _get_nc():
    if "nc" not in _NC_CACHE:
        import concourse.bacc as bacc
        import concourse.mybir as mybir
        from concourse import tile
        _NC_CACHE["nc"] = _build(bacc.Bacc, mybir, tile)
    return _NC_CACHE["nc"]


def kernel(x_samples, y_idx, W1, b1, W2, b2):
    from concourse.bass_utils import run_bass_kernel_spmd

    x = np.ascontiguousarray(np.asarray(x_samples, dtype=np.float32))
    y = np.asarray(y_idx).astype(np.int64).reshape(-1)
    W1 = np.ascontiguousarray(np.asarray(W1, dtype=np.float32))
    b1 = np.asarray(b1, dtype=np.float32).reshape(-1)
    W2 = np.ascontiguousarray(np.asarray(W2, dtype=np.float32))
    b2 = np.asarray(b2, dtype=np.float32).reshape(-1)

    # global label histogram + fully-folded bias term
    c = np.bincount(y, minlength=Y_DIM).astype(np.float32)
    cN = c / np.float32(N)
    beta = np.float32(b2 @ c) / np.float32(N)
    g_full = (b2[y] - beta).astype(np.float32)

    xT = np.ascontiguousarray(x.T)                                # [512, 8192]
    b1c = b1.reshape(KH, 128).T                                   # [128, 8]
    iot = np.arange(Y_DIM, dtype=np.float32).reshape(QY, 128).T   # [128, 4]
    cNc = cN.reshape(QY, 128).T                                   # [128, 4]
    onesv = np.ones((128, 1), dtype=np.float32)
    cst = np.ascontiguousarray(
        np.concatenate([b1c, iot, cNc, onesv], axis=1))           # [128, 17]

    in_maps = []
    for m in range(N_CORES):
        sl = slice(m * N_LOC, (m + 1) * N_LOC)
        y_loc = y[sl].astype(np.float32)
        in_maps.append({
            "xT": np.ascontiguousarray(xT[:, sl]),
            "W1": W1,
            "W2": W2,
            "cst": cst,
            "ybc": np.ascontiguousarray(
                np.broadcast_to(y_loc[None, :], (128, N_LOC))),
            "gv": np.ascontiguousarray(g_full[sl]).reshape(1, N_LOC),
        })

    nc = _get_nc()
    res = run_bass_kernel_spmd(nc, in_maps, core_ids=list(range(N_CORES)))
    return np.concatenate(
        [res.results[m]["out"].reshape(-1) for m in range(N_CORES)]
    ).astype(np.float32)


# revision 6
# speedup vs baseline: 1.0351x; 1.0351x over previous
"""Trainium2 Bass kernel for nn_CLUBCategorical (CLUB categorical loss).

Reference computation:
    h      = relu(x @ W1 + b1)              [N, H]
    logits = h @ W2 + b2                    [N, Y]
    logp   = log_softmax(logits, -1)        [N, Y]
    out[i] = logp[i, y_i] - mean_j logp[i, y_j]

Algebraic simplification used here: with c[y] = histogram(y_idx) the
log-softmax normalizer cancels between the positive and negative terms:

    out[i] = L[i, y_i] - (1/N) * (L[i, :] @ c) + (b2[y_i] - (b2 @ c)/N)

where L = relu(x @ W1 + b1) @ W2 (no bias, no softmax). On device this is
two dense matmuls plus a masked column reduction:

    out[i] = sum_y L[i, y] * (onehot(y_i)[y] - c[y]/N) + g[i]

Sharding: data-parallel over N. Each of the 8 cores handles 1024 rows and
gets the full W1/W2 plus the global label histogram (the "all-gather of
column labels" is precomputed on host into c). No collectives needed.

Device layout (per core, everything transposed so the contraction dim is
on SBUF partitions):
    phase 1: hT[j]   [128h, 1024r] = W1[k,jslice].T @ xT[k, rows]  (+b1, relu)
    phase 2: psum_l  [128y,  512r] = W2[j,qslice].T @ hT[j, rows]
             eqc     [128y,  512r] = (ybc == iota_q) - cN_q        (DVE)
             prod    = psum_l * eqc                                (DVE)
             out     += ones.T @ prod  (M=1 matmul, reduce over y) (PE)
Matmuls run in float32r (~TF32 precision, 4x faster than fp32 on PE).
"""

import numpy as np

N, X_DIM, Y_DIM, HIDDEN = 8192, 512, 512, 1024
N_CORES = 8
N_LOC = N // N_CORES          # 1024 rows per core
KX = X_DIM // 128             # 4  k-chunks, phase 1
KH = HIDDEN // 128            # 8  k-chunks, phase 2 / m-chunks, phase 1
QY = Y_DIM // 128             # 4  y-chunks, phase 2
RG = N_LOC // 512             # 2  row groups of 512

_NC_CACHE = {}


def _build(nc_cls, mybir, tile):
    mdt = mybir.dt
    f32 = mdt.float32
    F32R = mdt.float32r
    AF = mybir.ActivationFunctionType
    OP = mybir.AluOpType

    nc = nc_cls("TRN2", target_bir_lowering=False, debug=False,
                num_devices=N_CORES)

    # pre-arranged on host: xtD[n] rows = SBUF partitions, cols = (k, r)
    xtD = [nc.dram_tensor(f"xt{n}", [128, KX * 512], f32,
                          kind="ExternalInput") for n in range(RG)]
    w1D = [nc.dram_tensor(f"w1p{mp}", [128, KX * 256], f32,
                          kind="ExternalInput") for mp in range(KH // 2)]
    w2D = [nc.dram_tensor(f"w2p{h}", [128, 4 * Y_DIM], f32,
                          kind="ExternalInput") for h in range(2)]
    # packed constants: [b1c(8) | iot(4) | cNc(4) | ones(1)] = [128, 17]
    cst = nc.dram_tensor("cst", [128, KH + 2 * QY + 1], f32,
                         kind="ExternalInput")
    ybc = nc.dram_tensor("ybc", [128, N_LOC], f32, kind="ExternalInput")
    gv = nc.dram_tensor("gv", [1, N_LOC], f32, kind="ExternalInput")
    out = nc.dram_tensor("out", [1, N_LOC], f32, kind="ExternalOutput")

    with tile.TileContext(nc) as tc:
        with (
            tc.tile_pool(name="wgt", bufs=1) as wgt,
            tc.tile_pool(name="hp", bufs=1) as hp,
            tc.tile_pool(name="eqp", bufs=1) as eqp,
            tc.tile_pool(name="prp", bufs=4) as prp,
            tc.tile_pool(name="osb", bufs=1) as osb,
            tc.tile_pool(name="ph", bufs=3, space="PSUM") as ph,
            tc.tile_pool(name="pl", bufs=3, space="PSUM") as pl,
            tc.tile_pool(name="po", bufs=1, space="PSUM") as po,
        ):
            # DMA issue is serialized per DGE queue. Priority order:
            # phase-1 operands first (xT on sync HWDGE, W1 on scalar
            # HWDGE, interleaved by k so MMs start early); W2 follows W1
            # on the scalar queue (needed only when phase 2 starts);
            # small constants on the gpsimd SWDGE queue.
            cst_sb = wgt.tile([128, KH + 2 * QY + 1], F32R, tag="cst")
            nc.gpsimd.dma_start(cst_sb[:], cst.ap().bitcast(F32R))
            b1_sb = cst_sb[:, 0:KH].bitcast(f32)
            iot_sb = cst_sb[:, KH:KH + QY].bitcast(f32)
            cnc_sb = cst_sb[:, KH + QY:KH + 2 * QY].bitcast(f32)
            ones_sb = cst_sb[:, KH + 2 * QY:KH + 2 * QY + 1]
            g_sb = wgt.tile([1, N_LOC], f32, tag="g")
            nc.gpsimd.dma_start(g_sb[:], gv.ap())
            # Full-K column slices: one descriptor delivers ALL k-chunks
            # for a row-group (xT) or an m-pair (W1), so phase-1 groups
            # unlock after ~1.5MB instead of the full 4MB.
            # xt_sb[n] [128, KX, 512]: xT columns for row-group n, all K.
            xt_sb = []
            for n in range(RG):
                xtt = wgt.tile([128, KX * 512], F32R, tag=f"xt_{n}",
                               name=f"xt_{n}")
                nc.sync.dma_start(xtt[:], xtD[n].ap().bitcast(F32R))
                xt_sb.append(xtt)
            # w1_sb[mp] [128, KX, 256]: W1 columns for m-pair mp, all K.
            w1p_sb = []
            for mp in range(KH // 2):
                w1t = wgt.tile([128, KX * 256], F32R, tag=f"w1_{mp}",
                               name=f"w1_{mp}")
                nc.scalar.dma_start(w1t[:], w1D[mp].ap().bitcast(F32R))
                w1p_sb.append(w1t)

            def w1_slice(k, m):
                mp, mo = m // 2, m % 2
                return w1p_sb[mp][:, k * 256 + mo * 128:k * 256 + (mo + 1) * 128]

            def xt_slice(k, n):
                return xt_sb[n][:, k * 512:(k + 1) * 512]

            # W2 halves: one per HWDGE queue, right after the phase-1 loads
            w2p_sb = []
            for h in range(2):
                w2p = wgt.tile([128, 4 * Y_DIM], F32R, tag=f"w2p_{h}",
                               name=f"w2p_{h}")
                eng = nc.scalar if h == 0 else nc.sync
                eng.dma_start(w2p[:], w2D[h].ap().bitcast(F32R))
                w2p_sb.append(w2p)
            w2_sb = [w2p_sb[j // 4][:, (j % 4) * Y_DIM:(j % 4 + 1) * Y_DIM]
                     for j in range(KH)]
            ybc_sb = wgt.tile([128, N_LOC], f32, tag="ybc")
            nc.gpsimd.dma_start(ybc_sb[:], ybc.ap())

            # --- eqc masks (DVE; no matmul dependency, fills DVE idle time)
            eqc_sb = {}
            for n in range(RG):
                for q in range(QY):
                    e = eqp.tile([128, 512], f32, tag=f"eqc_{n}_{q}")
                    nc.vector.tensor_scalar(
                        e[:], ybc_sb[:, n * 512:(n + 1) * 512],
                        iot_sb[:, q:q + 1], cnc_sb[:, q:q + 1],
                        OP.is_equal, OP.subtract)
                    eqc_sb[(n, q)] = e

            # --- phase 1: hT[j] = relu(W1.T @ xT + b1) ---
            # n-outer so groups complete (and relu) as soon as the m-pair
            # column slice of W1 lands; relus stay spread out on ACT.
            hT = [hp.tile([128, N_LOC], F32R, tag=f"h_{j}", name=f"h_{j}")
                  for j in range(KH)]
            for n in range(RG):
                for m in range(KH):
                    psum = ph.tile([128, 512], f32)
                    for k in range(KX):
                        nc.tensor.matmul(
                            psum[:],
                            w1_slice(k, m),
                            xt_slice(k, n),
                            start=(k == 0), stop=(k == KX - 1))
                    nc.scalar.activation(
                        hT[m][:, n * 512:(n + 1) * 512], psum[:],
                        AF.Relu, bias=b1_sb[:, m:m + 1])

            # --- phase 2 ---
            pout = {n: po.tile([1, 512], f32, tag=f"po_{n}", name=f"po_{n}")
                    for n in range(RG)}
            pending = []  # delay ones-MMs one (n,q) step so PE never waits on DVE

            def flush_one():
                n, q, prod = pending.pop(0)
                nc.tensor.matmul(
                    pout[n][:], ones_sb, prod[:],
                    start=(q == 0), stop=(q == QY - 1))

            for n in range(RG):
                for q in range(QY):
                    psum_l = pl.tile([128, 512], f32)
                    for j in range(KH):
                        nc.tensor.matmul(
                            psum_l[:],
                            w2_sb[j][:, q * 128:(q + 1) * 128],
                            hT[j][:, n * 512:(n + 1) * 512],
                            start=(j == 0), stop=(j == KH - 1))
                    prod = prp.tile([128, 512], F32R)
                    nc.vector.tensor_tensor(
                        prod[:], psum_l[:], eqc_sb[(n, q)][:], OP.mult)
                    pending.append((n, q, prod))
                    if len(pending) >= 2:
                        flush_one()
            while pending:
                flush_one()

            # --- epilogue: add g, store (single output DMA) ---
            o = osb.tile([1, N_LOC], f32, tag="o")
            for n in range(RG):
                nc.vector.tensor_tensor(
                    o[:, n * 512:(n + 1) * 512], pout[n][:],
                    g_sb[:, n * 512:(n + 1) * 512], OP.add)
            nc.sync.dma_start(out.ap(), o[:])

    nc.compile()
    return nc


def _get_nc():
    if "nc" not in _NC_CACHE:
        import concourse.bacc as bacc
        import concourse.mybir as mybir
        from concourse import tile
        _NC_CACHE["nc"] = _build(bacc.Bacc, mybir, tile)
    return _NC_CACHE["nc"]


def kernel(x_samples, y_idx, W1, b1, W2, b2):
    from concourse.bass_utils import run_bass_kernel_spmd

    x = np.ascontiguousarray(np.asarray(x_samples, dtype=np.float32))
    y = np.asarray(y_idx).astype(np.int64).reshape(-1)
    W1 = np.ascontiguousarray(np.asarray(W1, dtype=np.float32))
    b1 = np.asarray(b1, dtype=np.float32).reshape(-1)
    W2 = np.ascontiguousarray(np.asarray(W2, dtype=np.float32))
    b2 = np.asarray(b2, dtype=np.float32).reshape(-1)

    # global label histogram + fully-folded bias term
    c = np.bincount(y, minlength=Y_DIM).astype(np.float32)
    cN = c / np.float32(N)
    beta = np.float32(b2 @ c) / np.float32(N)
    g_full = (b2[y] - beta).astype(np.float32)

    # device layouts: contiguous DMA descriptors, partition-major
    # w1_dev[mp][p, k*256+c] = W1[k*128+p, mp*256+c]
    w1_dev = np.ascontiguousarray(
        W1.reshape(KX, 128, KH // 2, 256).transpose(2, 1, 0, 3)
        .reshape(KH // 2, 128, KX * 256))
    # w2_dev[h][p, a*512+y] = W2[(h*4+a)*128+p, y]
    w2_dev = np.ascontiguousarray(
        W2.reshape(2, 4, 128, Y_DIM).transpose(0, 2, 1, 3)
        .reshape(2, 128, 4 * Y_DIM))
    b1c = b1.reshape(KH, 128).T                                   # [128, 8]
    iot = np.arange(Y_DIM, dtype=np.float32).reshape(QY, 128).T   # [128, 4]
    cNc = cN.reshape(QY, 128).T                                   # [128, 4]
    onesv = np.ones((128, 1), dtype=np.float32)
    cst = np.ascontiguousarray(
        np.concatenate([b1c, iot, cNc, onesv], axis=1))           # [128, 17]

    in_maps = []
    for m in range(N_CORES):
        sl = slice(m * N_LOC, (m + 1) * N_LOC)
        y_loc = y[sl].astype(np.float32)
        # xt_dev[n][p, k*512+r] = x[m*N_LOC + n*512+r, k*128+p]
        xt_dev = np.ascontiguousarray(
            x[sl].reshape(RG, 512, KX, 128).transpose(0, 3, 2, 1)
            .reshape(RG, 128, KX * 512))
        in_maps.append({
            **{f"xt{n}": xt_dev[n] for n in range(RG)},
            **{f"w1p{mp}": w1_dev[mp] for mp in range(KH // 2)},
            **{f"w2p{h}": w2_dev[h] for h in range(2)},
            "cst": cst,
            "ybc": np.ascontiguousarray(
                np.broadcast_to(y_loc[None, :], (128, N_LOC))),
            "gv": np.ascontiguousarray(g_full[sl]).reshape(1, N_LOC),
        })

    nc = _get_nc()
    res = run_bass_kernel_spmd(nc, in_maps, core_ids=list(range(N_CORES)))
    return np.concatenate(
        [res.results[m]["out"].reshape(-1) for m in range(N_CORES)]
    ).astype(np.float32)


# revision 7
# speedup vs baseline: 1.0499x; 1.0143x over previous
"""Trainium2 Bass kernel for nn_CLUBCategorical (CLUB categorical loss).

Reference computation:
    h      = relu(x @ W1 + b1)              [N, H]
    logits = h @ W2 + b2                    [N, Y]
    logp   = log_softmax(logits, -1)        [N, Y]
    out[i] = logp[i, y_i] - mean_j logp[i, y_j]

Algebraic simplification: with c[y] = histogram(y_idx), the log-softmax
normalizer cancels between the positive and negative terms:

    out[i] = L[i, y_i] - (1/N) * (L[i, :] @ c) + (b2[y_i] - (b2 @ c)/N)

where L = relu(x @ W1 + b1) @ W2 (no bias, no softmax). On device this is
two dense matmuls plus a masked column reduction:

    out[i] = sum_y L[i, y] * (onehot(y_i)[y] - c[y]/N) + g[i]

Sharding: data-parallel over N. Each of the 8 cores handles 1024 rows and
gets the full W1/W2 plus the global label histogram (the "all-gather of
column labels" is folded into c on the host). No collectives needed.

Device layout (per core; contraction dim always on SBUF partitions, all
operand layouts pre-arranged on host so every DMA is one contiguous
descriptor):
    phase 1: hT[m]  [128h, 1024r] = W1[:,mslice].T @ xT[:, rows] (+b1, relu)
    phase 2: psum_l [128y,  512r] = W2[:,qslice].T @ hT[:, rows]
             eqc    [128y,  512r] = (ybc == iota_q) - cN_q       (DVE)
             prod   = psum_l * eqc                               (DVE)
             out    += ones.T @ prod  (M=1 matmul reduces over y) (PE)
ybc is broadcast on device from a [1, rows] vector via a K=1 matmul.
Matmuls run in float32r (~2^-13 relative precision, 2x fp32 throughput).
DMA descriptors are interleaved across the two HWDGE queues (sync,
scalar) in phase-1 consumption order; constants ride the gpsimd SWDGE.
"""

import numpy as np

N, X_DIM, Y_DIM, HIDDEN = 8192, 512, 512, 1024
N_CORES = 8
N_LOC = N // N_CORES          # 1024 rows per core
KX = X_DIM // 128             # 4  k-chunks, phase 1
KH = HIDDEN // 128            # 8  k-chunks, phase 2 / m-chunks, phase 1
QY = Y_DIM // 128             # 4  y-chunks, phase 2
RG = N_LOC // 512             # 2  row groups of 512

_NC_CACHE = {}


def _build(nc_cls, mybir, tile):
    mdt = mybir.dt
    f32 = mdt.float32
    F32R = mdt.float32r
    AF = mybir.ActivationFunctionType
    OP = mybir.AluOpType

    nc = nc_cls("TRN2", target_bir_lowering=False, debug=False,
                num_devices=N_CORES)

    # xt{n}{a,b}: x rows for row-group n, partition-major, k-halves
    xtD = [[nc.dram_tensor(f"xt{n}{h}", [128, 2 * 512], f32,
                           kind="ExternalInput") for h in "ab"]
           for n in range(RG)]
    # w1p{mp}: W1 columns for hidden-pair mp, all K
    w1D = [nc.dram_tensor(f"w1p{mp}", [128, KX * 256], f32,
                          kind="ExternalInput") for mp in range(KH // 2)]
    # w2p{h}: W2 rows half h, partition-major
    w2D = [nc.dram_tensor(f"w2p{h}", [128, 4 * Y_DIM], f32,
                          kind="ExternalInput") for h in range(2)]
    # packed constants: [b1c(8) | iot(4) | cNc(4) | ones(1)] = [128, 17]
    cst = nc.dram_tensor("cst", [128, KH + 2 * QY + 1], f32,
                         kind="ExternalInput")
    o128 = nc.dram_tensor("o128", [1, 128], f32, kind="ExternalInput")
    yrow = nc.dram_tensor("yrow", [1, N_LOC], f32, kind="ExternalInput")
    gv = nc.dram_tensor("gv", [1, N_LOC], f32, kind="ExternalInput")
    out = nc.dram_tensor("out", [1, N_LOC], f32, kind="ExternalOutput")

    with tile.TileContext(nc) as tc:
        with (
            tc.tile_pool(name="wgt", bufs=1) as wgt,
            tc.tile_pool(name="hp", bufs=1) as hp,
            tc.tile_pool(name="eqp", bufs=1) as eqp,
            tc.tile_pool(name="prp", bufs=4) as prp,
            tc.tile_pool(name="osb", bufs=1) as osb,
            tc.tile_pool(name="ps", bufs=3, space="PSUM") as ps,
            tc.tile_pool(name="po", bufs=1, space="PSUM") as po,
        ):
            # --- constants on the gpsimd SWDGE queue (tiny) ---
            cst_sb = wgt.tile([128, KH + 2 * QY + 1], F32R, tag="cst")
            nc.gpsimd.dma_start(cst_sb[:], cst.ap().bitcast(F32R))
            b1_sb = cst_sb[:, 0:KH].bitcast(f32)
            iot_sb = cst_sb[:, KH:KH + QY].bitcast(f32)
            cnc_sb = cst_sb[:, KH + QY:KH + 2 * QY].bitcast(f32)
            ones_sb = cst_sb[:, KH + 2 * QY:KH + 2 * QY + 1]
            o128_sb = wgt.tile([1, 128], F32R, tag="o128")
            nc.gpsimd.dma_start(o128_sb[:], o128.ap().bitcast(F32R))
            yrow_sb = wgt.tile([1, N_LOC], F32R, tag="yrow")
            nc.gpsimd.dma_start(yrow_sb[:], yrow.ap().bitcast(F32R))
            g_sb = wgt.tile([1, N_LOC], f32, tag="g")
            nc.gpsimd.dma_start(g_sb[:], gv.ap())

            # --- big loads, interleaved across both HWDGE queues in
            # phase-1 consumption order ---
            xt_sb = [wgt.tile([128, KX * 512], F32R, tag=f"xt_{n}",
                              name=f"xt_{n}") for n in range(RG)]
            w1p_sb = [wgt.tile([128, KX * 256], F32R, tag=f"w1_{mp}",
                               name=f"w1_{mp}") for mp in range(KH // 2)]
            w2p_sb = [wgt.tile([128, 4 * Y_DIM], F32R, tag=f"w2p_{h}",
                               name=f"w2p_{h}") for h in range(2)]
            # sync:   xt0a, w1p1, w1p3, xt1a, w2p1
            # scalar: xt0b, w1p0, w1p2, xt1b, w2p0
            nc.sync.dma_start(xt_sb[0][:, 0:1024],
                              xtD[0][0].ap().bitcast(F32R))
            nc.scalar.dma_start(xt_sb[0][:, 1024:2048],
                                xtD[0][1].ap().bitcast(F32R))
            nc.sync.dma_start(w1p_sb[1][:], w1D[1].ap().bitcast(F32R))
            nc.scalar.dma_start(w1p_sb[0][:], w1D[0].ap().bitcast(F32R))
            nc.sync.dma_start(w1p_sb[3][:], w1D[3].ap().bitcast(F32R))
            nc.scalar.dma_start(w1p_sb[2][:], w1D[2].ap().bitcast(F32R))
            nc.sync.dma_start(xt_sb[1][:, 0:1024],
                              xtD[1][0].ap().bitcast(F32R))
            nc.scalar.dma_start(xt_sb[1][:, 1024:2048],
                                xtD[1][1].ap().bitcast(F32R))
            nc.sync.dma_start(w2p_sb[1][:], w2D[1].ap().bitcast(F32R))
            nc.scalar.dma_start(w2p_sb[0][:], w2D[0].ap().bitcast(F32R))
            w2_sb = [w2p_sb[j // 4][:, (j % 4) * Y_DIM:(j % 4 + 1) * Y_DIM]
                     for j in range(KH)]

            def w1_slice(k, m):
                mp, mo = m // 2, m % 2
                return w1p_sb[mp][:, k * 256 + mo * 128:
                                  k * 256 + (mo + 1) * 128]

            def xt_slice(k, n):
                return xt_sb[n][:, k * 512:(k + 1) * 512]

            # --- ybc broadcast: K=1 matmul replicates yrow across
            # partitions; eqc masks read it straight from PSUM ---
            eqc_sb = {}
            for n in range(RG):
                yb = ps.tile([128, 512], f32, tag="yb", bufs=RG,
                             name=f"yb{n}")
                nc.tensor.matmul(
                    yb[:], o128_sb[:],
                    yrow_sb[:, n * 512:(n + 1) * 512],
                    start=True, stop=True)
                for q in range(QY):
                    e = eqp.tile([128, 512], f32, tag=f"eqc_{n}_{q}",
                                 name=f"eqc_{n}_{q}")
                    nc.vector.tensor_scalar(
                        e[:], yb[:], iot_sb[:, q:q + 1], cnc_sb[:, q:q + 1],
                        OP.is_equal, OP.subtract)
                    eqc_sb[(n, q)] = e

            # --- phase 1: hT[m] = relu(W1.T @ xT + b1) ---
            # n-outer; groups complete as soon as the W1 column pair lands
            hT = [hp.tile([128, N_LOC], F32R, tag=f"h_{j}", name=f"h_{j}")
                  for j in range(KH)]
            for n in range(RG):
                for m in range(KH):
                    psum = ps.tile([128, 512], f32)
                    for k in range(KX):
                        nc.tensor.matmul(
                            psum[:], w1_slice(k, m), xt_slice(k, n),
                            start=(k == 0), stop=(k == KX - 1))
                    nc.scalar.activation(
                        hT[m][:, n * 512:(n + 1) * 512], psum[:],
                        AF.Relu, bias=b1_sb[:, m:m + 1])

            # --- phase 2 ---
            pout = {n: po.tile([1, 512], f32, tag=f"po_{n}", name=f"po_{n}")
                    for n in range(RG)}
            pending = []  # delay ones-MMs so PE never waits on DVE prod

            def flush_one():
                n, q, prod = pending.pop(0)
                nc.tensor.matmul(
                    pout[n][:], ones_sb, prod[:],
                    start=(q == 0), stop=(q == QY - 1))

            for n in range(RG):
                for q in range(QY):
                    psum_l = ps.tile([128, 512], f32, tag="psum",
                                     name=f"pl_{n}_{q}")
                    for j in range(KH):
                        nc.tensor.matmul(
                            psum_l[:],
                            w2_sb[j][:, q * 128:(q + 1) * 128],
                            hT[j][:, n * 512:(n + 1) * 512],
                            start=(j == 0), stop=(j == KH - 1))
                    prod = prp.tile([128, 512], F32R)
                    nc.vector.tensor_tensor(
                        prod[:], psum_l[:], eqc_sb[(n, q)][:], OP.mult)
                    pending.append((n, q, prod))
                    if len(pending) >= 2:
                        flush_one()
            while pending:
                flush_one()

            # --- epilogue: add g, store (single output DMA) ---
            o = osb.tile([1, N_LOC], f32, tag="o")
            for n in range(RG):
                nc.vector.tensor_tensor(
                    o[:, n * 512:(n + 1) * 512], pout[n][:],
                    g_sb[:, n * 512:(n + 1) * 512], OP.add)
            nc.sync.dma_start(out.ap(), o[:])

    nc.compile()
    return nc


def _get_nc():
    if "nc" not in _NC_CACHE:
        import concourse.bacc as bacc
        import concourse.mybir as mybir
        from concourse import tile
        _NC_CACHE["nc"] = _build(bacc.Bacc, mybir, tile)
    return _NC_CACHE["nc"]


def kernel(x_samples, y_idx, W1, b1, W2, b2):
    from concourse.bass_utils import run_bass_kernel_spmd

    x = np.ascontiguousarray(np.asarray(x_samples, dtype=np.float32))
    y = np.asarray(y_idx).astype(np.int64).reshape(-1)
    W1 = np.ascontiguousarray(np.asarray(W1, dtype=np.float32))
    b1 = np.asarray(b1, dtype=np.float32).reshape(-1)
    W2 = np.ascontiguousarray(np.asarray(W2, dtype=np.float32))
    b2 = np.asarray(b2, dtype=np.float32).reshape(-1)

    # global label histogram + fully-folded bias term
    c = np.bincount(y, minlength=Y_DIM).astype(np.float32)
    cN = c / np.float32(N)
    beta = np.float32(b2 @ c) / np.float32(N)
    g_full = (b2[y] - beta).astype(np.float32)

    # device layouts: every DMA is one contiguous descriptor
    # w1_dev[mp][p, k*256+c] = W1[k*128+p, mp*256+c]
    w1_dev = np.ascontiguousarray(
        W1.reshape(KX, 128, KH // 2, 256).transpose(2, 1, 0, 3)
        .reshape(KH // 2, 128, KX * 256))
    # w2_dev[h][p, a*512+y] = W2[(h*4+a)*128+p, y]
    w2_dev = np.ascontiguousarray(
        W2.reshape(2, 4, 128, Y_DIM).transpose(0, 2, 1, 3)
        .reshape(2, 128, 4 * Y_DIM))
    b1c = b1.reshape(KH, 128).T                                   # [128, 8]
    iot = np.arange(Y_DIM, dtype=np.float32).reshape(QY, 128).T   # [128, 4]
    cNc = cN.reshape(QY, 128).T                                   # [128, 4]
    onesv = np.ones((128, 1), dtype=np.float32)
    cst = np.ascontiguousarray(
        np.concatenate([b1c, iot, cNc, onesv], axis=1))           # [128, 17]
    o128 = np.ones((1, 128), dtype=np.float32)

    in_maps = []
    for m in range(N_CORES):
        sl = slice(m * N_LOC, (m + 1) * N_LOC)
        # xt_dev[n][p, k*512+r] = x[m*N_LOC + n*512+r, k*128+p]
        xt_dev = np.ascontiguousarray(
            x[sl].reshape(RG, 512, KX, 128).transpose(0, 3, 2, 1)
            .reshape(RG, 128, KX * 512))
        im = {
            **{f"w1p{mp}": w1_dev[mp] for mp in range(KH // 2)},
            **{f"w2p{h}": w2_dev[h] for h in range(2)},
            "cst": cst,
            "o128": o128,
            "yrow": np.ascontiguousarray(
                y[sl].astype(np.float32)).reshape(1, N_LOC),
            "gv": np.ascontiguousarray(g_full[sl]).reshape(1, N_LOC),
        }
        for n in range(RG):
            im[f"xt{n}a"] = np.ascontiguousarray(xt_dev[n][:, 0:1024])
            im[f"xt{n}b"] = np.ascontiguousarray(xt_dev[n][:, 1024:2048])
        in_maps.append(im)

    nc = _get_nc()
    res = run_bass_kernel_spmd(nc, in_maps, core_ids=list(range(N_CORES)))
    return np.concatenate(
        [res.results[m]["out"].reshape(-1) for m in range(N_CORES)]
    ).astype(np.float32)


# revision 8
# speedup vs baseline: 1.0824x; 1.0309x over previous
"""Trainium2 Bass kernel for nn_CLUBCategorical (CLUB categorical loss).

Reference computation:
    h      = relu(x @ W1 + b1)              [N, H]
    logits = h @ W2 + b2                    [N, Y]
    logp   = log_softmax(logits, -1)        [N, Y]
    out[i] = logp[i, y_i] - mean_j logp[i, y_j]

Algebraic simplification: with c[y] = histogram(y_idx), the log-softmax
normalizer cancels between the positive and negative terms:

    out[i] = L[i, y_i] - (1/N) * (L[i, :] @ c) + (b2[y_i] - (b2 @ c)/N)

where L = relu(x @ W1 + b1) @ W2 (no bias, no softmax). On device this is
two dense matmuls plus a masked column reduction:

    out[i] = sum_y L[i, y] * (onehot(y_i)[y] - c[y]/N) + g[i]

Sharding: data-parallel over N. Each of the 8 cores handles 1024 rows and
gets the full W1/W2 plus the global label histogram (the "all-gather of
column labels" is folded into c on the host). No collectives needed.

Device layout (per core; contraction dim always on SBUF partitions, all
operand layouts pre-arranged on host so every DMA is one contiguous
descriptor):
    phase 1: hT[m]  [128h, 1024r] = W1[:,mslice].T @ xT[:, rows] (+b1, relu)
    phase 2: psum_l [128y,  512r] = W2[:,qslice].T @ hT[:, rows]
             eqc    [128y,  512r] = (ybc == iota_q) - cN_q       (DVE)
             prod   = psum_l * eqc                               (DVE)
             out    += ones.T @ prod  (M=1 matmul reduces over y) (PE)
ybc is broadcast on device from a [1, rows] vector via a K=1 matmul.
Matmuls run in float32r (~2^-13 relative precision, 2x fp32 throughput).
DMA descriptors are interleaved across the two HWDGE queues (sync,
scalar) in phase-1 consumption order; constants ride the gpsimd SWDGE.
"""

import numpy as np

N, X_DIM, Y_DIM, HIDDEN = 8192, 512, 512, 1024
N_CORES = 8
N_LOC = N // N_CORES          # 1024 rows per core
KX = X_DIM // 128             # 4  k-chunks, phase 1
KH = HIDDEN // 128            # 8  k-chunks, phase 2 / m-chunks, phase 1
QY = Y_DIM // 128             # 4  y-chunks, phase 2
RG = N_LOC // 512             # 2  row groups of 512

_NC_CACHE = {}


def _build(nc_cls, mybir, tile):
    mdt = mybir.dt
    f32 = mdt.float32
    F32R = mdt.float32r
    AF = mybir.ActivationFunctionType
    OP = mybir.AluOpType

    nc = nc_cls("TRN2", target_bir_lowering=False, debug=False,
                num_devices=N_CORES)

    # xt{n}{a,b}: x rows for row-group n, partition-major, k-halves
    xtD = [[nc.dram_tensor(f"xt{n}{h}", [128, 2 * 512], f32,
                           kind="ExternalInput") for h in "ab"]
           for n in range(RG)]
    # w1p{mp}: W1 columns for hidden-pair mp, all K
    w1D = [nc.dram_tensor(f"w1p{mp}", [128, KX * 256], f32,
                          kind="ExternalInput") for mp in range(KH // 2)]
    # w2p{h}: W2 rows half h, partition-major
    w2D = [nc.dram_tensor(f"w2p{h}", [128, 4 * Y_DIM], f32,
                          kind="ExternalInput") for h in range(2)]
    # packed constants: [b1c(8) | iot(4) | cNc(4) | ones(1)] = [128, 17]
    cst = nc.dram_tensor("cst", [128, KH + 2 * QY + 1], f32,
                         kind="ExternalInput")
    o128 = nc.dram_tensor("o128", [1, 128], f32, kind="ExternalInput")
    yrow = nc.dram_tensor("yrow", [1, N_LOC], f32, kind="ExternalInput")
    gv = nc.dram_tensor("gv", [1, N_LOC], f32, kind="ExternalInput")
    out = nc.dram_tensor("out", [1, N_LOC], f32, kind="ExternalOutput")

    with tile.TileContext(nc) as tc:
        with (
            tc.tile_pool(name="wgt", bufs=1) as wgt,
            tc.tile_pool(name="hp", bufs=1) as hp,
            tc.tile_pool(name="eqp", bufs=1) as eqp,
            tc.tile_pool(name="prp", bufs=4) as prp,
            tc.tile_pool(name="osb", bufs=1) as osb,
            tc.tile_pool(name="ps", bufs=3, space="PSUM") as ps,
            tc.tile_pool(name="po", bufs=1, space="PSUM") as po,
        ):
            # --- tiny tensors first on the HWDGE queues (they gate the
            # PE warmup matmuls and the eqc masks) ---
            yrow_sb = wgt.tile([1, N_LOC], F32R, tag="yrow")
            nc.sync.dma_start(yrow_sb[:], yrow.ap().bitcast(F32R))
            o128_sb = wgt.tile([1, 128], F32R, tag="o128")
            nc.sync.dma_start(o128_sb[:], o128.ap().bitcast(F32R))
            cst_sb = wgt.tile([128, KH + 2 * QY + 1], F32R, tag="cst")
            nc.scalar.dma_start(cst_sb[:], cst.ap().bitcast(F32R))
            b1_sb = cst_sb[:, 0:KH].bitcast(f32)
            iot_sb = cst_sb[:, KH:KH + QY].bitcast(f32)
            cnc_sb = cst_sb[:, KH + QY:KH + 2 * QY].bitcast(f32)
            ones_sb = cst_sb[:, KH + 2 * QY:KH + 2 * QY + 1]
            g_sb = wgt.tile([1, N_LOC], f32, tag="g")
            nc.gpsimd.dma_start(g_sb[:], gv.ap())

            # --- big loads, interleaved across both HWDGE queues in
            # phase-1 consumption order ---
            xt_sb = [wgt.tile([128, KX * 512], F32R, tag=f"xt_{n}",
                              name=f"xt_{n}") for n in range(RG)]
            w1p_sb = [wgt.tile([128, KX * 256], F32R, tag=f"w1_{mp}",
                               name=f"w1_{mp}") for mp in range(KH // 2)]
            w2p_sb = [wgt.tile([128, 4 * Y_DIM], F32R, tag=f"w2p_{h}",
                               name=f"w2p_{h}") for h in range(2)]
            # sync:   xt0a, w1p1, w1p3, xt1a, w2p1
            # scalar: xt0b, w1p0, w1p2, xt1b, w2p0
            nc.sync.dma_start(xt_sb[0][:, 0:1024],
                              xtD[0][0].ap().bitcast(F32R))
            nc.scalar.dma_start(xt_sb[0][:, 1024:2048],
                                xtD[0][1].ap().bitcast(F32R))
            nc.sync.dma_start(w1p_sb[1][:], w1D[1].ap().bitcast(F32R))
            nc.scalar.dma_start(w1p_sb[0][:], w1D[0].ap().bitcast(F32R))
            nc.sync.dma_start(w1p_sb[3][:], w1D[3].ap().bitcast(F32R))
            nc.scalar.dma_start(w1p_sb[2][:], w1D[2].ap().bitcast(F32R))
            nc.sync.dma_start(xt_sb[1][:, 0:1024],
                              xtD[1][0].ap().bitcast(F32R))
            nc.scalar.dma_start(xt_sb[1][:, 1024:2048],
                                xtD[1][1].ap().bitcast(F32R))
            nc.sync.dma_start(w2p_sb[1][:], w2D[1].ap().bitcast(F32R))
            nc.scalar.dma_start(w2p_sb[0][:], w2D[0].ap().bitcast(F32R))
            w2_sb = [w2p_sb[j // 4][:, (j % 4) * Y_DIM:(j % 4 + 1) * Y_DIM]
                     for j in range(KH)]

            def w1_slice(k, m):
                mp, mo = m // 2, m % 2
                return w1p_sb[mp][:, k * 256 + mo * 128:
                                  k * 256 + (mo + 1) * 128]

            def xt_slice(k, n):
                return xt_sb[n][:, k * 512:(k + 1) * 512]

            # --- PE warmup: ~10 throwaway K=1 matmuls into the pout
            # bank (idle until phase 2), gated only by the tiny yrow/o128
            # loads. Keeps the PE busy through a full HAM activity window
            # so phase 1 starts at the 2.4GHz clock instead of 1.2. ---
            pout = {n: po.tile([1, 512], f32, tag=f"po_{n}", name=f"po_{n}")
                    for n in range(RG)}
            for _ in range(10):
                nc.tensor.matmul(pout[0][:], o128_sb[:, 0:1],
                                 yrow_sb[:, 0:512], start=True, stop=True)

            # --- ybc broadcast: K=1 matmul replicates yrow across
            # partitions; eqc masks read it straight from PSUM ---
            eqc_sb = {}
            for n in range(RG):
                yb = ps.tile([128, 512], f32, tag="yb", bufs=RG,
                             name=f"yb{n}")
                nc.tensor.matmul(
                    yb[:], o128_sb[:],
                    yrow_sb[:, n * 512:(n + 1) * 512],
                    start=True, stop=True)
                for q in range(QY):
                    e = eqp.tile([128, 512], f32, tag=f"eqc_{n}_{q}",
                                 name=f"eqc_{n}_{q}")
                    nc.vector.tensor_scalar(
                        e[:], yb[:], iot_sb[:, q:q + 1], cnc_sb[:, q:q + 1],
                        OP.is_equal, OP.subtract)
                    eqc_sb[(n, q)] = e

            # --- phase 1: hT[m] = relu(W1.T @ xT + b1) ---
            # n-outer; groups complete as soon as the W1 column pair lands
            hT = [hp.tile([128, N_LOC], F32R, tag=f"h_{j}", name=f"h_{j}")
                  for j in range(KH)]
            for n in range(RG):
                for m in range(KH):
                    psum = ps.tile([128, 512], f32)
                    for k in range(KX):
                        nc.tensor.matmul(
                            psum[:], w1_slice(k, m), xt_slice(k, n),
                            start=(k == 0), stop=(k == KX - 1))
                    nc.scalar.activation(
                        hT[m][:, n * 512:(n + 1) * 512], psum[:],
                        AF.Relu, bias=b1_sb[:, m:m + 1])

            # --- phase 2 ---
            pending = []  # delay ones-MMs so PE never waits on DVE prod

            def flush_one():
                n, q, prod = pending.pop(0)
                nc.tensor.matmul(
                    pout[n][:], ones_sb, prod[:],
                    start=(q == 0), stop=(q == QY - 1))

            for n in range(RG):
                for q in range(QY):
                    psum_l = ps.tile([128, 512], f32, tag="psum",
                                     name=f"pl_{n}_{q}")
                    for j in range(KH):
                        nc.tensor.matmul(
                            psum_l[:],
                            w2_sb[j][:, q * 128:(q + 1) * 128],
                            hT[j][:, n * 512:(n + 1) * 512],
                            start=(j == 0), stop=(j == KH - 1))
                    prod = prp.tile([128, 512], F32R)
                    nc.vector.tensor_tensor(
                        prod[:], psum_l[:], eqc_sb[(n, q)][:], OP.mult)
                    pending.append((n, q, prod))
                    if len(pending) >= 2:
                        flush_one()
            while pending:
                flush_one()

            # --- epilogue: add g, store (single output DMA) ---
            o = osb.tile([1, N_LOC], f32, tag="o")
            for n in range(RG):
                nc.vector.tensor_tensor(
                    o[:, n * 512:(n + 1) * 512], pout[n][:],
                    g_sb[:, n * 512:(n + 1) * 512], OP.add)
            nc.sync.dma_start(out.ap(), o[:])

    nc.compile()
    return nc


def _get_nc():
    if "nc" not in _NC_CACHE:
        import concourse.bacc as bacc
        import concourse.mybir as mybir
        from concourse import tile
        _NC_CACHE["nc"] = _build(bacc.Bacc, mybir, tile)
    return _NC_CACHE["nc"]


def kernel(x_samples, y_idx, W1, b1, W2, b2):
    from concourse.bass_utils import run_bass_kernel_spmd

    x = np.ascontiguousarray(np.asarray(x_samples, dtype=np.float32))
    y = np.asarray(y_idx).astype(np.int64).reshape(-1)
    W1 = np.ascontiguousarray(np.asarray(W1, dtype=np.float32))
    b1 = np.asarray(b1, dtype=np.float32).reshape(-1)
    W2 = np.ascontiguousarray(np.asarray(W2, dtype=np.float32))
    b2 = np.asarray(b2, dtype=np.float32).reshape(-1)

    # global label histogram + fully-folded bias term
    c = np.bincount(y, minlength=Y_DIM).astype(np.float32)
    cN = c / np.float32(N)
    beta = np.float32(b2 @ c) / np.float32(N)
    g_full = (b2[y] - beta).astype(np.float32)

    # device layouts: every DMA is one contiguous descriptor
    # w1_dev[mp][p, k*256+c] = W1[k*128+p, mp*256+c]
    w1_dev = np.ascontiguousarray(
        W1.reshape(KX, 128, KH // 2, 256).transpose(2, 1, 0, 3)
        .reshape(KH // 2, 128, KX * 256))
    # w2_dev[h][p, a*512+y] = W2[(h*4+a)*128+p, y]
    w2_dev = np.ascontiguousarray(
        W2.reshape(2, 4, 128, Y_DIM).transpose(0, 2, 1, 3)
        .reshape(2, 128, 4 * Y_DIM))
    b1c = b1.reshape(KH, 128).T                                   # [128, 8]
    iot = np.arange(Y_DIM, dtype=np.float32).reshape(QY, 128).T   # [128, 4]
    cNc = cN.reshape(QY, 128).T                                   # [128, 4]
    onesv = np.ones((128, 1), dtype=np.float32)
    cst = np.ascontiguousarray(
        np.concatenate([b1c, iot, cNc, onesv], axis=1))           # [128, 17]
    o128 = np.ones((1, 128), dtype=np.float32)

    in_maps = []
    for m in range(N_CORES):
        sl = slice(m * N_LOC, (m + 1) * N_LOC)
        # xt_dev[n][p, k*512+r] = x[m*N_LOC + n*512+r, k*128+p]
        xt_dev = np.ascontiguousarray(
            x[sl].reshape(RG, 512, KX, 128).transpose(0, 3, 2, 1)
            .reshape(RG, 128, KX * 512))
        im = {
            **{f"w1p{mp}": w1_dev[mp] for mp in range(KH // 2)},
            **{f"w2p{h}": w2_dev[h] for h in range(2)},
            "cst": cst,
            "o128": o128,
            "yrow": np.ascontiguousarray(
                y[sl].astype(np.float32)).reshape(1, N_LOC),
            "gv": np.ascontiguousarray(g_full[sl]).reshape(1, N_LOC),
        }
        for n in range(RG):
            im[f"xt{n}a"] = np.ascontiguousarray(xt_dev[n][:, 0:1024])
            im[f"xt{n}b"] = np.ascontiguousarray(xt_dev[n][:, 1024:2048])
        in_maps.append(im)

    nc = _get_nc()
    res = run_bass_kernel_spmd(nc, in_maps, core_ids=list(range(N_CORES)))
    return np.concatenate(
        [res.results[m]["out"].reshape(-1) for m in range(N_CORES)]
    ).astype(np.float32)
